# revision 1
# baseline (speedup 1.0000x reference)
"""Trainium2 Bass kernel for nn_BaseRecommender (masked top-k recommendation).

Strategy (hardcoded, self-contained):
  - Shard the item embedding table column-wise (item dim) across 8 cores:
    12500 items/core, zero-padded to 13312 = 13 matmul chunks x 1024.
  - Replicate u_e = all_embed[user_list] (gathered + transposed on host).
  - Per core: float32r matmul (64-dim contraction, 4x faster than fp32 on the
    PE, ~1e-4 relative noise) -> PSUM [128 rows x 1024]; the scalar engine
    copies PSUM -> SBUF (the DVE reads PSUM ~8x slower than SBUF, so the
    copy pays for itself); DVE max/max_index extract per-3328-item-chunk
    top-8 values + indices.  8 row tiles x (13 copy chunks / 4 DVE chunks).
  - Host: exact fp32 scores for global item columns [0, 1024) (the only
    range the reference ever masks, since it keeps only item_idx < BATCH),
    exact recompute of every device candidate's score, merge, and re-select
    the global top-k.  A guard recomputes any chunk whose 8th returned
    candidate could still reach the row's top-20 (covers both the top-8
    truncation and the f32r noise), so the result is exact.
"""

import os
import sys

import numpy as np

try:
    import concourse  # noqa: F401
except ImportError:
    for _p in ("/opt/trn_rl_repo", os.path.expanduser("~/.axon_site/_ro/trn_rl_repo")):
        if os.path.isdir(_p):
            sys.path.insert(0, _p)
            try:
                import concourse  # noqa: F401

                break
            except ImportError:
                sys.path.remove(_p)

N_USERS = 100000
N_ITEMS = 100000
EMB = 64
BATCH = 1024
K = 20
NEG = -100000.0
NCORES = 8
ISHARD = N_ITEMS // NCORES  # 12500 items per core
PCH = 1024  # matmul/psum chunk (columns)
NPCH = 13  # psum chunks per core
IPAD = NPCH * PCH  # 13312
DVCH = 3328  # DVE top-8 chunk (= IPAD / 4)
NDV = IPAD // DVCH  # 4 DVE chunks
ROWT = 128
NROWT = BATCH // ROWT  # 8 row tiles
NCAND = NDV * 8  # 32 candidates per row per core
HOST_COLS = 1024  # item columns [0, HOST_COLS) are scored on host (mask range)
F32R_EPS = 1e-3  # relative guard margin for float32r matmul noise

_compiled = None


def _build_bass(loop_n=1):
    """Build the per-core Bass program. loop_n > 1 repeats the compute loop
    (hardware For_i) for differential HW timing; loads happen once."""
    from concourse import bacc
    import concourse.mybir as mybir
    from concourse.tile import TileContext

    F32 = mybir.dt.float32
    F32R = mybir.dt.float32r

    nc = bacc.Bacc("TRN2", target_bir_lowering=False, debug=False, num_devices=NCORES)
    u_t = nc.dram_tensor("u_t", [EMB, BATCH], F32R, kind="ExternalInput")
    i_t = nc.dram_tensor("i_t", [EMB, IPAD], F32R, kind="ExternalInput")
    cv = nc.dram_tensor("cv", [BATCH, NCAND], F32, kind="ExternalOutput")
    ci = nc.dram_tensor("ci", [BATCH, NCAND], mybir.dt.uint32, kind="ExternalOutput")

    with TileContext(nc) as tc:
        with (
            tc.tile_pool(name="consts", bufs=1) as consts,
            tc.tile_pool(name="psum", bufs=4, space="PSUM") as psum,
            tc.tile_pool(name="scores", bufs=2) as scores,
            tc.tile_pool(name="cand", bufs=2) as cand,
        ):
            u_sb = consts.tile([EMB, BATCH], F32R, tag="u_sb")
            nc.sync.dma_start(u_sb[:], u_t[:])
            i_sb = []
            for c in range(NPCH):
                t = consts.tile([EMB, PCH], F32R, tag=f"i_sb{c}")
                nc.sync.dma_start(t[:], i_t[:, c * PCH : (c + 1) * PCH])
                i_sb.append(t)

            def body():
                for rt in range(NROWT):
                    s_sb = scores.tile([ROWT, IPAD], F32, tag="s_sb")
                    cv_t = cand.tile([ROWT, NCAND], F32, tag="cv_t")
                    ci_t = cand.tile([ROWT, NCAND], mybir.dt.uint32, tag="ci_t")
                    lhs = u_sb[:, rt * ROWT : (rt + 1) * ROWT]
                    for c in range(NPCH):
                        ps = psum.tile([ROWT, PCH], F32, tag="ps")
                        nc.tensor.matmul(
                            ps[:, 0:512], lhs, i_sb[c][:, 0:512], start=True, stop=True
                        )
                        nc.tensor.matmul(
                            ps[:, 512:1024],
                            lhs,
                            i_sb[c][:, 512:1024],
                            start=True,
                            stop=True,
                        )
                        nc.scalar.copy(s_sb[:, c * PCH : (c + 1) * PCH], ps[:])
                    for d in range(NDV):
                        seg = s_sb[:, d * DVCH : (d + 1) * DVCH]
                        nc.vector.max(cv_t[:, d * 8 : (d + 1) * 8], seg)
                        nc.vector.max_index(
                            ci_t[:, d * 8 : (d + 1) * 8],
                            cv_t[:, d * 8 : (d + 1) * 8],
                            seg,
                        )
                    nc.sync.dma_start(cv[rt * ROWT : (rt + 1) * ROWT, :], cv_t[:])
                    nc.sync.dma_start(ci[rt * ROWT : (rt + 1) * ROWT, :], ci_t[:])

            if loop_n == 1:
                body()
            else:
                with tc.For_i(0, loop_n, 1):
                    body()

    nc.compile()
    return nc


def _get_compiled():
    global _compiled
    if _compiled is None:
        _compiled = _build_bass()
    return _compiled


def run_device(u_t, i_t_shards, trace=False, **kwargs):
    from concourse.bass_utils import run_bass_kernel_spmd

    nc = _get_compiled()
    in_maps = [{"u_t": u_t, "i_t": i_t_shards[s]} for s in range(NCORES)]
    return run_bass_kernel_spmd(nc, in_maps, list(range(NCORES)), trace=trace, **kwargs)


def make_device_inputs(all_embed, user_list):
    all_embed = np.asarray(all_embed, dtype=np.float32)
    user_list = np.asarray(user_list)
    u_e = all_embed[user_list.astype(np.int64)]  # [BATCH, EMB]
    i_e = all_embed[N_USERS:]  # [N_ITEMS, EMB]
    u_t = np.ascontiguousarray(u_e.T)  # [EMB, BATCH]
    i_t_shards = []
    for s in range(NCORES):
        sh = np.zeros((EMB, IPAD), dtype=np.float32)
        sh[:, :ISHARD] = i_e[s * ISHARD : (s + 1) * ISHARD].T
        i_t_shards.append(sh)
    return u_e, i_e, u_t, i_t_shards


def _mask_host_scores(s0, pos_pad):
    """Reference masking semantics on the host-scored region: only valid
    positives with local item index < BATCH (== HOST_COLS) are masked."""
    pos_pad = np.asarray(pos_pad)
    item_idx = pos_pad.astype(np.int64) - N_USERS
    valid = (pos_pad >= 0) & (item_idx < HOST_COLS)
    r, c = np.nonzero(valid)
    np.minimum.at(s0, (r, item_idx[r, c]), np.float32(NEG))
    return s0


def postprocess(results, u_e, i_e, pos_pad):
    """Merge per-core per-chunk top-8 candidates into the exact global top-K."""
    raw_v = np.empty((NCORES, BATCH, NCAND), dtype=np.float32)
    dev_g = np.empty((NCORES, BATCH, NCAND), dtype=np.int64)
    dev_ok = np.empty((NCORES, BATCH, NCAND), dtype=bool)
    for s in range(NCORES):
        raw_v[s] = results[s]["cv"]
        local = (np.arange(NCAND, dtype=np.int64) // 8) * DVCH + results[s][
            "ci"
        ].astype(np.int64)
        dev_g[s] = s * ISHARD + local
        dev_ok[s] = (local < ISHARD) & (dev_g[s] >= HOST_COLS)

    # Exact scores for every valid device candidate (removes f32r noise).
    cand_g = dev_g.transpose(1, 0, 2).reshape(BATCH, NCORES * NCAND)
    cand_ok = dev_ok.transpose(1, 0, 2).reshape(BATCH, NCORES * NCAND)
    safe_g = np.where(cand_ok, cand_g, 0)
    cand_v = np.einsum("re,rce->rc", u_e, i_e[safe_g], optimize=True).astype(np.float32)
    cand_v[~cand_ok] = -np.inf
    cand_g = np.where(cand_ok, cand_g, -1)

    # Host-exact scores for the maskable region (global item cols [0, 1024)).
    s0 = u_e @ i_e[:HOST_COLS].T  # [BATCH, HOST_COLS] float32
    s0 = _mask_host_scores(s0, pos_pad)
    hp = np.argpartition(-s0, K, axis=1)[:, :K]
    hv = np.take_along_axis(s0, hp, axis=1).astype(np.float32)

    all_v = np.concatenate([hv, cand_v], axis=1)  # [BATCH, K + 256]
    all_g = np.concatenate([hp.astype(np.int64), cand_g], axis=1)

    # Vectorized selection on exact values.
    order = np.argsort(-all_v, axis=1, kind="stable")[:, : K + 1]
    rows = np.arange(BATCH)[:, None]
    sel_v = all_v[rows, order]
    v20 = sel_v[:, K - 1]

    # Guard: chunk's 8th returned (f32r-noisy) value + margin can still reach
    # the row's 20th -> that chunk may hide candidates; recompute it exactly.
    slot8 = raw_v.reshape(NCORES, BATCH, NDV, 8)[:, :, :, 7]
    scale = np.maximum(np.abs(sel_v[:, 0]), 1.0)  # [BATCH]
    margin = F32R_EPS * scale
    trig = slot8 + margin[None, :, None] >= v20[None, :, None]
    tie = sel_v[:, K - 1] == sel_v[:, K]
    careful = set(np.nonzero(trig.any(axis=(0, 2)) | tie)[0].tolist())

    out_idx = np.empty((BATCH, K), dtype=np.int64)
    out_val = np.empty((BATCH, K), dtype=np.float32)

    top_g = all_g[rows, order[:, :K]]
    top_v = sel_v[:, :K]
    for r in range(BATCH):
        o = np.lexsort((top_g[r], -top_v[r]))
        out_idx[r] = top_g[r][o]
        out_val[r] = top_v[r][o]

    for r in careful:
        vals = list(all_v[r])
        idxs = list(all_g[r])
        recomputed = set()
        while True:
            vv = np.asarray(vals, dtype=np.float64)
            gg = np.asarray(idxs, dtype=np.int64)
            o = np.lexsort((gg, -vv))[:K]
            tg, tv = gg[o], vv[o]
            r20 = tv[-1]
            trig_r = [
                (s, d)
                for s in range(NCORES)
                for d in range(NDV)
                if (s, d) not in recomputed and slot8[s, r, d] + margin[r] >= r20
            ]
            if not trig_r:
                break
            for s, d in trig_r:
                recomputed.add((s, d))
                # invalidate the chunk's original candidates (superseded by
                # the full-chunk recompute; avoids duplicate indices)
                base = K + s * NCAND + d * 8
                for j in range(base, base + 8):
                    vals[j] = -np.inf
                    idxs[j] = -1
                lo = s * ISHARD + d * DVCH
                hi = min(lo + DVCH, (s + 1) * ISHARD)
                lo_eff = max(lo, HOST_COLS)
                if lo_eff >= hi:
                    continue
                sc = (i_e[lo_eff:hi] @ u_e[r]).astype(np.float32)
                vals.extend(sc.tolist())
                idxs.extend(range(lo_eff, hi))
        out_idx[r] = tg
        out_val[r] = tv.astype(np.float32)

    return out_idx.astype(np.int32) + N_USERS, out_val


def kernel(all_embed, pos_pad, user_list, k):
    pos_pad = np.asarray(pos_pad)
    k = int(k)
    assert k == K, f"kernel hardcoded for k={K}, got {k}"
    u_e, i_e, u_t, i_t_shards = make_device_inputs(all_embed, user_list)
    res = run_device(u_t, i_t_shards)
    return postprocess(res.results, u_e, i_e, pos_pad)

